# revision 42
# baseline (speedup 1.0000x reference)
"""Trainium2 Bass kernel: pre-LN single-head causal attention + residual.

Reference computation (B=4, S=2048, H=K=2048, fp32):
    xn = LayerNorm(x) * gamma + beta
    q,k,v = xn @ qkv (split)
    out = causal_softmax(q k^T / sqrt(K)) @ v @ o_proj + x

Sharding: 8 cores = 4 batches x 2 query-halves, folded-causal split into 4
classes of 256 query rows with key extents 512*(e+1) (identical program on
all cores; per-core behavior comes only from the permuted inputs/masks).

All big matmuls run in fp8 (e4m3) with DoubleRow perf mode: 256 contraction
rows per pass = 2x bf16 PE throughput. Weights are host-scaled by 64 into
fp8 range; scales are folded into PSUM evictions (q,k,v: 1/64; out^T: 1/16;
the softmax-denominator "ones" vector is 4.0 so recip = 1/(4*sums) exactly
compensates oT/16 @ (64*wo) = 4 * out @ wo).

xnT token-column order is LOCAL: this core's own 8 chunks first (position
chunks 0,1,4,5,8,9,12,13), then the peer-destined 8 -- so the k/q matmul
moving slices are contiguous 512-wide.
"""
import os
import sys

import numpy as np

sys.path.insert(0, "/opt/trn_rl_repo")


def _install_ntff_hook():
    """Register the axon NTFF profile hook bass_utils expects (the image's
    antenv package lacks axon_hooks); degrades to no-op when unavailable."""
    import types
    if "antenv.axon_hooks" in sys.modules:
        return
    try:
        from trn_agent_boot.trn_boot import _ntff_profile_via_ctypes
        hook = _ntff_profile_via_ctypes("/opt/axon/libaxon_pjrt.so")
    except Exception:
        hook = None
    m = types.ModuleType("antenv.axon_hooks")
    m.get_axon_ntff_profile_hook = lambda: hook
    sys.modules["antenv.axon_hooks"] = m


_install_ntff_hook()

import ml_dtypes  # noqa: E402
import concourse.bass as bass  # noqa: E402
import concourse.tile as tile  # noqa: E402
from concourse import bacc, mybir  # noqa: E402
from concourse.bass_utils import run_bass_kernel_spmd  # noqa: E402

F32 = mybir.dt.float32
BF16 = mybir.dt.bfloat16
F8 = mybir.dt.float8e4
AF = mybir.ActivationFunctionType
OP = mybir.AluOpType
DR = mybir.MatmulPerfMode.DoubleRow

B, S, H, KEY = 4, 2048, 2048, 2048
NCHK = 16                 # 128-row chunks per sequence
EPS = 1e-5
SCALE = 1.0 / float(np.sqrt(KEY))
WSCALE = 64.0             # host multiplier on weights before fp8 cast
ABASE = [0, 4, 12, 24]    # attn^T tile base index per class
ATOT = 40                 # total k-chunk tiles across classes
PC = [0, 1, 4, 5, 8, 9, 12, 13]   # position chunks holding this core's q rows
OC = [2, 3, 6, 7, 10, 11, 14, 15]  # peer-destined position chunks


def perm_chunks(h):
    out = []
    for e in range(4):
        out += [4 * e + 2 * h, 4 * e + 2 * h + 1,
                4 * e + 2 * (1 - h), 4 * e + 2 * (1 - h) + 1]
    return out


def build():
    nc = bacc.Bacc("TRN2", target_bir_lowering=False, debug=False, num_devices=8)

    x_d = nc.dram_tensor("x", [S, H], F32, kind="ExternalInput")
    # weights pre-transposed on host for contiguous DMA:
    # wq/wk: [kc, p, hc, j]; wv/wo: [p, hc|kvc, j] (all fp8, x64 scaled)
    wq_d = nc.dram_tensor("wq", [NCHK, 128, NCHK, 128], F8, kind="ExternalInput")
    wk_d = nc.dram_tensor("wk", [NCHK, 128, NCHK, 128], F8, kind="ExternalInput")
    wv_d = nc.dram_tensor("wv", [128, NCHK, KEY], F8, kind="ExternalInput")
    wo_d = nc.dram_tensor("wo", [128, NCHK, H], F8, kind="ExternalInput")
    gamma_d = nc.dram_tensor("gamma", [128, NCHK], F32, kind="ExternalInput")
    beta_d = nc.dram_tensor("beta", [128, NCHK], F32, kind="ExternalInput")
    mask_d = nc.dram_tensor("mask", [4, 4, 128, 256], F8, kind="ExternalInput")
    y_d = nc.dram_tensor("y", [1024, H], F32, kind="ExternalOutput")
    DBG = bool(os.environ.get("K_DEBUG"))
    if DBG:
        dbg_s = nc.dram_tensor("dbg_s", [4, 256], F32, kind="ExternalOutput")
        dbg_q = nc.dram_tensor("dbg_q", [NCHK, 128, 1024], F8, kind="ExternalOutput")
        dbg_k = nc.dram_tensor("dbg_k", [2, 2, NCHK, 128, 512], F8,
                               kind="ExternalOutput")
        dbg_v = nc.dram_tensor("dbg_v", [2, 8, 128, KEY], F8, kind="ExternalOutput")
        dbg_a = nc.dram_tensor("dbg_a", [ATOT, 128, 256], F8, kind="ExternalOutput")
        dbg_o = nc.dram_tensor("dbg_o", [NCHK, 128, 1024], F8, kind="ExternalOutput")
        dbg_xn = nc.dram_tensor("dbg_xn", [NCHK, 128, S], F8, kind="ExternalOutput")
    ssp_d = nc.dram_tensor("ssp", [4, 256], F32, kind="Internal")
    # combined spill buffers per half: rows 0-3 = v chunks, rows 4-7 = k^T
    # tiles packed as [4, 128, 2048] (ks[kc] at row 4+kc//4, cols (kc%4)*512)
    vka_d = nc.dram_tensor("vka", [8, 128, KEY], F8, kind="Internal")
    vkb_d = nc.dram_tensor("vkb", [8, 128, KEY], F8, kind="Internal")
    vkap_d = nc.dram_tensor("vkap", [2, 8, 128, KEY], F8, kind="Internal")
    vkbp_d = nc.dram_tensor("vkbp", [2, 8, 128, KEY], F8, kind="Internal")
    GROUPS = [[2 * p, 2 * p + 1] for p in range(4)]

    ident = nc.inline_tensor(np.eye(128).astype(ml_dtypes.bfloat16), name="ident")

    with tile.TileContext(nc) as tc:
        with (
            tc.tile_pool(name="small", bufs=1) as small,
            tc.tile_pool(name="p_main", bufs=1) as p_main,
        ):
            recip = small.tile([128, 8], F32)         # 1/(4*sums) per q-chunk
            gcol = small.tile([128, NCHK], F32)       # gamma, [p, hc]
            bcol = small.tile([128, NCHK], F32)       # beta
            ones = small.tile([128, 1], F8)           # 4.0 (denominator scale)
            id16_sb = small.tile([128, 128], BF16)

            xnT = p_main.tile([128, NCHK, S], F8)  # x_norm^T [hid_p, hc, local tok]
            qT = p_main.tile([128, NCHK, 1024], F8)  # q^T [key_p, kc, class-packed q]

            nc.vector.memset(ones[:], 4.0)

            # ---------- A0: LN + transpose + v + k (interleaved) ----------
            with (
                nc.named_scope("ln_transpose"),
                tc.tile_pool(name="a0x", bufs=2) as a0x,
                tc.tile_pool(name="a0xp", bufs=6) as a0xp,
                tc.tile_pool(name="a0s", bufs=4) as a0s,
                tc.tile_pool(name="wv", bufs=1) as wvp,
                tc.tile_pool(name="vst", bufs=2) as vst,
                tc.tile_pool(name="kst", bufs=2) as kst,
                tc.tile_pool(name="pp_tr", bufs=2, space=bass.MemorySpace.PSUM) as pp_tr,
                tc.tile_pool(name="pp_v", bufs=2, space=bass.MemorySpace.PSUM) as pp_v,
                tc.tile_pool(name="pp_k", bufs=2, space=bass.MemorySpace.PSUM) as pp_k,
                tc.tile_pool(name="pp_q", bufs=2, space=bass.MemorySpace.PSUM) as pp_q,
            ):
                # resident weights, one big DMA each (scalar queue, so the
                # sync queue keeps feeding x chunks without head-of-line wait)
                wv_sb = wvp.tile([128, NCHK, KEY], F8)
                nc.scalar.dma_start(wv_sb[:], wv_d[:])
                wk_sb = wvp.tile([128, NCHK, NCHK, 128], F8)   # [p, kc, hc, j]
                nc.scalar.dma_start(wk_sb[:],
                                    wk_d.rearrange("kc p hc j -> p kc hc j"))
                wq_sb = wvp.tile([128, NCHK, NCHK, 128], F8)
                nc.scalar.dma_start(wq_sb[:],
                                    wq_d.rearrange("kc p hc j -> p kc hc j"))

                def v_pass(lc):
                    """v for own local chunk lc -> combined spill row lc%4."""
                    vs = vst.tile([128, KEY], F8, tag="vs")
                    for kvt in range(4):
                        ps = pp_v.tile([128, 512], F32, tag="v")
                        for hp in range(NCHK // 2):
                            nc.tensor.matmul(
                                ps[:],
                                xnT[:, 2 * hp:2 * hp + 2, lc * 128:(lc + 1) * 128],
                                wv_sb[:, 2 * hp:2 * hp + 2,
                                      kvt * 512:(kvt + 1) * 512],
                                start=(hp == 0), stop=(hp == NCHK // 2 - 1),
                                perf_mode=DR)
                        nc.scalar.activation(vs[:, kvt * 512:(kvt + 1) * 512],
                                             ps[:], AF.Identity,
                                             scale=1.0 / WSCALE)
                    vk = [vka_d, vkb_d][lc // 4]
                    nc.scalar.dma_start(vk[lc % 4][:], vs[:])

                def k_pass(half):
                    """k^T for local tok chunks 4*half..4*half+3 (xnT cols
                    half*512..half*512+512); spills into the combined buffer
                    then gathers v+k for this half in ONE AllGather."""
                    vk = [vka_d, vkb_d][half]
                    gout = [vkap_d, vkbp_d][half]
                    for kc in range(NCHK):
                        psk = pp_k.tile([128, 512], F32, tag="k")
                        for hp in range(NCHK // 2):
                            nc.tensor.matmul(
                                psk[:], wk_sb[:, kc, 2 * hp:2 * hp + 2, :],
                                xnT[:, 2 * hp:2 * hp + 2,
                                    half * 512:(half + 1) * 512],
                                start=(hp == 0), stop=(hp == NCHK // 2 - 1),
                                perf_mode=DR)
                        ks = kst.tile([128, 512], F8, tag="ks")
                        nc.scalar.activation(ks[:], psk[:], AF.Identity,
                                             scale=1.0 / WSCALE)
                        nc.scalar.dma_start(
                            vk[4 + kc // 4, :, (kc % 4) * 512:(kc % 4 + 1) * 512],
                            ks[:])
                    nc.gpsimd.collective_compute(
                        "AllGather", OP.bypass, replica_groups=GROUPS,
                        ins=[vk.ap().opt()], outs=[gout.ap().opt()])

                def q_pass(g):
                    """q^T for xnT cols g*512..(g+1)*512 (classes 2g, 2g+1)."""
                    for kc in range(NCHK):
                        psq = pp_q.tile([128, 512], F32, tag="q")
                        for hp in range(NCHK // 2):
                            nc.tensor.matmul(
                                psq[:], wq_sb[:, kc, 2 * hp:2 * hp + 2, :],
                                xnT[:, 2 * hp:2 * hp + 2, g * 512:(g + 1) * 512],
                                start=(hp == 0), stop=(hp == NCHK // 2 - 1),
                                perf_mode=DR)
                        nc.scalar.activation(qT[:, kc, g * 512:(g + 1) * 512],
                                             psq[:], AF.Identity,
                                             scale=1.0 / WSCALE)

                for tg in range(4):
                    xps = []
                    for i in range(4):
                        tci = tg * 4 + i
                        x_t = a0x.tile([128, H], F32, tag="x")
                        nc.sync.dma_start(x_t[:], x_d[tci * 128:(tci + 1) * 128, :])
                        st = a0s.tile([128, 4, 6], F32, tag="st")
                        for j in range(4):
                            nc.vector.bn_stats(st[:, j, :], x_t[:, j * 512:(j + 1) * 512])
                        ag = a0s.tile([128, 2], F32, tag="ag")
                        nc.vector.bn_aggr(ag[:], st[:])
                        veps = a0s.tile([128, 1], F32, tag="veps")
                        nc.vector.tensor_scalar_add(veps[:], ag[:, 1:2], EPS)
                        sq = a0s.tile([128, 1], F32, tag="sq")
                        nc.scalar.sqrt(sq[:], veps[:])
                        rstd = a0s.tile([128, 1], F32, tag="rstd")
                        nc.vector.reciprocal(rstd[:], sq[:])
                        nmr = a0s.tile([128, 1], F32, tag="nmr")
                        nc.vector.tensor_scalar(nmr[:], ag[:, 0:1], rstd[:], -1.0,
                                                OP.mult, OP.mult)
                        xp = a0xp.tile([128, H], BF16, tag="xp")
                        nc.vector.tensor_scalar(xp[:], x_t[:], rstd[:], nmr[:],
                                                OP.mult, OP.add)
                        xps.append(xp)
                    if tg == 0:
                        # small loads after tg0's x DMAs: x keeps HBM priority
                        nc.sync.dma_start(id16_sb[:], ident[:])
                        nc.sync.dma_start(gcol[:], gamma_d[:])
                        nc.sync.dma_start(bcol[:], beta_d[:])
                    # position chunks tg*4+i: i=0,1 own -> local cols
                    # (2*tg+i)*128; i=2,3 peer -> local cols 1024+(2*tg+i-2)*128
                    for hc in range(NCHK):
                        ps = pp_tr.tile([128, 512], BF16, tag="tr")
                        for i in range(4):
                            nc.tensor.transpose(ps[:, i * 128:(i + 1) * 128],
                                                xps[i][:, hc * 128:(hc + 1) * 128],
                                                id16_sb[:])
                        # one strided eviction: ps cols 0:256 -> own local
                        # cols, 256:512 -> peer cols (base 1024)
                        xr = xnT[:, hc, :].rearrange("p (g t) -> p g t", g=2)
                        nc.scalar.activation(
                            xr[:, :, (2 * tg) * 128:(2 * tg + 2) * 128],
                            ps[:], AF.Identity,
                            bias=bcol[:, hc:hc + 1], scale=gcol[:, hc:hc + 1])
                    # v redistributed: tg0 none (x DMAs get HBM priority while
                    # weights stream); each half's v spills complete before
                    # its k_pass emits the combined v+k AllGather
                    for lc in {0: [], 1: [0, 1, 2, 3], 2: [4, 5], 3: [6, 7]}[tg]:
                        v_pass(lc)
                    if tg == 1 or tg == 3:
                        k_pass(tg // 2)
                        q_pass(tg // 2)

            # ---------- pools for q^T, attn^T, out^T, v-resident ----------
            p_bc0 = tc.tile_pool(name="p_bc", bufs=1)
            p_bc = p_bc0.__enter__()
            aT = p_bc.tile([128, ATOT, 256], F8)      # attn^T tiles
            oT = p_bc.tile([128, NCHK, 1024], F8)     # out^T [kv_p, kvc, q]
            vts = p_bc.tile([128, 16, KEY], F8)       # v^T-source [tok_p, slot, kv]
            wo_sb = p_bc.tile([128, NCHK, H], F8)     # o_proj weights [kv_p, kvc, j]

            # ---------- B+C interleaved: scores -> attn^T; out^T per class ----
            with (
                nc.named_scope("scores"),
                tc.tile_pool(name="bk", bufs=3) as bk,
                tc.tile_pool(name="bm", bufs=4) as bm,
                tc.tile_pool(name="bs", bufs=4) as bs,
                tc.tile_pool(name="pp_s", bufs=2, space=bass.MemorySpace.PSUM) as pp_s,
                tc.tile_pool(name="pp_o", bufs=2, space=bass.MemorySpace.PSUM) as pp_o,
                tc.tile_pool(name="pp_sum", bufs=4,
                             space=bass.MemorySpace.PSUM) as pp_sum,
            ):
                # v resident: [tok_p, slot(r*8+lc), kv]; first half available
                # as soon as the half-0 combined AllGather lands
                for r in range(2):
                    nc.sync.dma_start(
                        vts[:, r * 8:r * 8 + 4, :],
                        vkap_d[r, 0:4].rearrange("lc p j -> p lc j"))
                # o_proj weights prefetch (used in D)
                nc.scalar.dma_start(wo_sb[:], wo_d[:])

                ps_sums = [pp_sum.tile([1, 256], F32, tag="sum", name=f"psum{e}")
                           for e in range(4)]
                pending = []

                def emit_c(e):
                    """out^T class e: needs aT slots lc<=2e+1 (both ranks)."""
                    for kvc in range(NCHK):
                        ps_o = pp_o.tile([128, 256], F32, tag="o")
                        steps = [(r, j) for r in range(2) for j in range(e + 1)]
                        for si, (r, j) in enumerate(steps):
                            nc.tensor.matmul(
                                ps_o[:],
                                vts[:, r * 8 + 2 * j:r * 8 + 2 * j + 2,
                                    kvc * 128:(kvc + 1) * 128],
                                aT[:, ABASE[e] + r * 2 * (e + 1) + 2 * j:
                                   ABASE[e] + r * 2 * (e + 1) + 2 * j + 2, :],
                                start=(si == 0), stop=(si == len(steps) - 1),
                                perf_mode=DR)
                        nc.scalar.activation(oT[:, kvc, e * 256:(e + 1) * 256],
                                             ps_o[:], AF.Identity, scale=1.0 / 16)

                # slot (r, lc): rank r's local tok chunk lc; tile-grouped by
                # (half, r) with 4 lc each. Class e consumes lc < 2*(e+1).
                for half in range(2):
                    src = [vkap_d, vkbp_d][half]
                    if half == 1:
                        for r in range(2):
                            nc.sync.dma_start(
                                vts[:, r * 8 + 4:r * 8 + 8, :],
                                vkbp_d[r, 0:4].rearrange("lc p j -> p lc j"))
                    for r in range(2):
                        # k^T tiles: rows 4-7 of the combined gather, packed
                        # [q][p][(k j)] with kc = q*4 + k
                        ktq4 = bk.tile([128, 4, 4, 512], F8, tag="kt")
                        nc.sync.dma_start(
                            ktq4[:], src[r, 4:8].rearrange(
                                "q p (k j) -> p q k j", j=512))
                        ktq = ktq4[:].rearrange("p q k j -> p (q k) j")
                        for lcc in range(4):
                            lc = half * 4 + lcc
                            this_round = []
                            for e in range(lc // 2, 4):
                                ps_s = pp_s.tile([128, 256], F32, tag="s")
                                for kp in range(NCHK // 2):
                                    nc.tensor.matmul(
                                        ps_s[:],
                                        ktq[:, 2 * kp:2 * kp + 2,
                                            lcc * 128:(lcc + 1) * 128],
                                        qT[:, 2 * kp:2 * kp + 2,
                                           e * 256:(e + 1) * 256],
                                        start=(kp == 0), stop=(kp == NCHK // 2 - 1),
                                        perf_mode=DR)
                                dst = aT[:, ABASE[e] + r * 2 * (e + 1) + lc, :]
                                if lc // 2 == e:
                                    tmp = bs.tile([128, 256], F8, tag="exps")
                                    nc.scalar.activation(tmp[:], ps_s[:], AF.Exp,
                                                         scale=SCALE)
                                    mt = bm.tile([128, 256], F8, tag="mask")
                                    nc.sync.dma_start(mt[:], mask_d[e, r * 2 + lc % 2])
                                    nc.vector.tensor_mul(dst, tmp[:], mt[:])
                                else:
                                    nc.scalar.activation(dst, ps_s[:], AF.Exp,
                                                         scale=SCALE)
                                this_round.append((e, (r, lc), dst))
                            for e, pos, src2 in pending:
                                nc.tensor.matmul(ps_sums[e][:], ones[:, 0:1], src2,
                                                 start=(pos == (0, 0)),
                                                 stop=(pos == (1, 2 * e + 1)))
                            pending = this_round
                            if r == 1 and lc % 2 == 1:
                                ecl = lc // 2    # class ecl complete
                                for e, pos, src2 in pending:
                                    nc.tensor.matmul(
                                        ps_sums[e][:], ones[:, 0:1], src2,
                                        start=(pos == (0, 0)),
                                        stop=(pos == (1, 2 * e + 1)))
                                pending = []
                                emit_c(ecl)
                for e in range(4):
                    srow = bs.tile([1, 256], F32, tag="srow", name=f"srow{e}")
                    nc.scalar.copy(srow[:], ps_sums[e][:])
                    nc.scalar.dma_start(ssp_d[e], srow[:])
                for e in range(4):
                    scol = bs.tile([128, 2], F32, tag="scol", name=f"scol{e}")
                    nc.sync.dma_start(scol[:],
                                      ssp_d[e].rearrange("(j p) -> p j", p=128))
                    nc.vector.reciprocal(recip[:, 2 * e:2 * e + 2], scol[:])

            if DBG:
                nc.sync.dma_start(dbg_s[:], ssp_d[:])
                for r in range(2):
                    for q in range(4):
                        nc.sync.dma_start(
                            dbg_k[0, r, q * 4:(q + 1) * 4],
                            vkap_d[r, 4 + q].rearrange("p (k j) -> k p j", j=512))
                        nc.sync.dma_start(
                            dbg_k[1, r, q * 4:(q + 1) * 4],
                            vkbp_d[r, 4 + q].rearrange("p (k j) -> k p j", j=512))
                    nc.sync.dma_start(dbg_v[r, 0:4], vkap_d[r, 0:4])
                    nc.sync.dma_start(dbg_v[r, 4:8], vkbp_d[r, 0:4])
                for kc in range(NCHK):
                    nc.sync.dma_start(dbg_q[kc], qT[:, kc, :])
                    nc.sync.dma_start(dbg_o[kc], oT[:, kc, :])
                    nc.sync.dma_start(dbg_xn[kc], xnT[:, kc, :])
                for t in range(ATOT):
                    nc.sync.dma_start(dbg_a[t], aT[:, t, :])

            # ---------- D: y = diag(recip) (oT^T @ Wo) + x ----------
            with (
                nc.named_scope("o_proj"),
                tc.tile_pool(name="dx", bufs=6) as dx,
                tc.tile_pool(name="dy", bufs=4) as dy,
                tc.tile_pool(name="pp_y", bufs=6, space=bass.MemorySpace.PSUM) as pp_y,
            ):
                for ht in range(4):
                    for qg in range(2):
                        psy = [pp_y.tile([128, 512], F32, tag="y", name=f"psy{i}")
                               for i in range(4)]
                        for kp in range(NCHK // 2):
                            for i in range(4):
                                qc = qg * 4 + i
                                nc.tensor.matmul(
                                    psy[i][:],
                                    oT[:, 2 * kp:2 * kp + 2, qc * 128:(qc + 1) * 128],
                                    wo_sb[:, 2 * kp:2 * kp + 2,
                                          ht * 512:(ht + 1) * 512],
                                    start=(kp == 0), stop=(kp == NCHK // 2 - 1),
                                    perf_mode=DR)
                        for i in range(4):
                            qc = qg * 4 + i
                            xres = dx.tile([128, 512], F32, tag="xr")
                            nc.sync.dma_start(xres[:],
                                              x_d[PC[qc] * 128:(PC[qc] + 1) * 128,
                                                  ht * 512:(ht + 1) * 512])
                            ysb = dy.tile([128, 512], F32, tag="y")
                            nc.vector.scalar_tensor_tensor(
                                ysb[:], psy[i][:], recip[:, qc:qc + 1], xres[:],
                                OP.mult, OP.add)
                            eng = nc.scalar if i % 2 else nc.gpsimd
                            eng.dma_start(y_d[qc * 128:(qc + 1) * 128,
                                              ht * 512:(ht + 1) * 512],
                                          ysb[:])
            p_bc0.__exit__(None, None, None)
    nc.compile()
    return nc


_NC_CACHE = None


def _get_nc():
    global _NC_CACHE
    if _NC_CACHE is None:
        _NC_CACHE = build()
    return _NC_CACHE


def make_in_maps(x, qkv, o_proj, gamma, beta):
    qkv = np.asarray(qkv, dtype=np.float32)
    o_proj = np.asarray(o_proj, dtype=np.float32)
    F8NP = ml_dtypes.float8_e4m3

    def prep_qk(w):  # [H, KEY] -> [kc, p, hc, j] fp8 (x64)
        t = (WSCALE * w).reshape(NCHK, 128, NCHK, 128)  # [hc, p, kc, j]
        return np.ascontiguousarray(t.transpose(2, 1, 0, 3)).astype(F8NP)

    def prep_vo(w):  # [H, KEY] -> [p, hc, j] fp8 (x64)
        t = (WSCALE * w).reshape(NCHK, 128, KEY)        # [hc, p, j]
        return np.ascontiguousarray(t.transpose(1, 0, 2)).astype(F8NP)

    wq8 = prep_qk(qkv[:, :KEY])
    wk8 = prep_qk(qkv[:, KEY:2 * KEY])
    wv8 = prep_vo(qkv[:, 2 * KEY:])
    wo8 = prep_vo(o_proj)
    gamma = np.ascontiguousarray(
        np.asarray(gamma, dtype=np.float32).reshape(NCHK, 128).T)
    beta = np.ascontiguousarray(
        np.asarray(beta, dtype=np.float32).reshape(NCHK, 128).T)
    in_maps, metas = [], []
    for c in range(8):
        b, h = c // 2, c % 2
        P = perm_chunks(h)
        ti = np.concatenate([np.arange(pc * 128, pc * 128 + 128) for pc in P])
        x_perm = np.ascontiguousarray(x[b][ti], dtype=np.float32)
        # mask[e][2r+j]: k tok-slot (rank r, quad e, j) holds true chunk
        # 4e+2r+j; q col c of class e is true row ti[512e+c].
        mask = np.zeros((4, 4, 128, 256), dtype=F8NP)
        for e in range(4):
            qp = ti[512 * e:512 * e + 256]
            for r in range(2):
                for j in range(2):
                    kp = (4 * e + 2 * r + j) * 128 + np.arange(128)
                    mask[e, 2 * r + j] = (kp[:, None] <= qp[None, :]).astype(F8NP)
        in_maps.append({"x": x_perm, "wq": wq8, "wk": wk8, "wv": wv8,
                        "wo": wo8, "gamma": gamma, "beta": beta, "mask": mask})
        metas.append((b, ti))
    return in_maps, metas


def gather(results, metas, dtype):
    out = np.empty((B, S, H), dtype=dtype)
    qpos = np.concatenate([np.arange(512 * e, 512 * e + 256) for e in range(4)])
    for c, (b, ti) in enumerate(metas):
        out[b][ti[qpos]] = results[c]["y"]
    return out


def kernel(x, qkv, o_proj, gamma, beta, _trace=False):
    x = np.asarray(x, dtype=np.float32)
    nc = _get_nc()
    in_maps, metas = make_in_maps(x, qkv, o_proj, gamma, beta)
    res = run_bass_kernel_spmd(nc, in_maps, core_ids=list(range(8)), trace=_trace)
    out = gather(res.results, metas, np.float32)
    if _trace:
        kernel.last_result = res
    return out


# revision 45
# speedup vs baseline: 1.1055x; 1.1055x over previous
"""Trainium2 Bass kernel: pre-LN single-head causal attention + residual.

Reference computation (B=4, S=2048, H=K=2048, fp32):
    xn = LayerNorm(x) * gamma + beta
    q,k,v = xn @ qkv (split)
    out = causal_softmax(q k^T / sqrt(K)) @ v @ o_proj + x

Sharding: 8 cores = 4 batches x 2 query-halves, folded-causal split into 4
classes of 256 query rows with key extents 512*(e+1) (identical program on
all cores; per-core behavior comes only from the permuted inputs/masks).

All big matmuls run in fp8 (e4m3) with DoubleRow perf mode: 256 contraction
rows per pass = 2x bf16 PE throughput. Weights are host-scaled by 64 into
fp8 range; scales are folded into PSUM evictions (q,k,v: 1/64; out^T: 1/16;
the softmax-denominator "ones" vector is 4.0 so recip = 1/(4*sums) exactly
compensates oT/16 @ (64*wo) = 4 * out @ wo).

xnT token-column order is LOCAL: this core's own 8 chunks first (position
chunks 0,1,4,5,8,9,12,13), then the peer-destined 8 -- so the k/q matmul
moving slices are contiguous 512-wide.
"""
import os
import sys

import numpy as np

sys.path.insert(0, "/opt/trn_rl_repo")


def _install_ntff_hook():
    """Register the axon NTFF profile hook bass_utils expects (the image's
    antenv package lacks axon_hooks); degrades to no-op when unavailable."""
    import types
    if "antenv.axon_hooks" in sys.modules:
        return
    try:
        from trn_agent_boot.trn_boot import _ntff_profile_via_ctypes
        hook = _ntff_profile_via_ctypes("/opt/axon/libaxon_pjrt.so")
    except Exception:
        hook = None
    m = types.ModuleType("antenv.axon_hooks")
    m.get_axon_ntff_profile_hook = lambda: hook
    sys.modules["antenv.axon_hooks"] = m


_install_ntff_hook()

import ml_dtypes  # noqa: E402
import concourse.bass as bass  # noqa: E402
import concourse.tile as tile  # noqa: E402
from concourse import bacc, mybir  # noqa: E402
from concourse.bass_utils import run_bass_kernel_spmd  # noqa: E402

F32 = mybir.dt.float32
BF16 = mybir.dt.bfloat16
F8 = mybir.dt.float8e4
AF = mybir.ActivationFunctionType
OP = mybir.AluOpType
DR = mybir.MatmulPerfMode.DoubleRow

B, S, H, KEY = 4, 2048, 2048, 2048
NCHK = 16                 # 128-row chunks per sequence
EPS = 1e-5
SCALE = 1.0 / float(np.sqrt(KEY))
WSCALE = 64.0             # host multiplier on weights before fp8 cast
ABASE = [0, 4, 12, 24]    # attn^T tile base index per class
ATOT = 40                 # total k-chunk tiles across classes
PC = [0, 1, 4, 5, 8, 9, 12, 13]   # position chunks holding this core's q rows
OC = [2, 3, 6, 7, 10, 11, 14, 15]  # peer-destined position chunks


def perm_chunks(h):
    out = []
    for e in range(4):
        out += [4 * e + 2 * h, 4 * e + 2 * h + 1,
                4 * e + 2 * (1 - h), 4 * e + 2 * (1 - h) + 1]
    return out


def build():
    nc = bacc.Bacc("TRN2", target_bir_lowering=False, debug=False, num_devices=8)

    x_d = nc.dram_tensor("x", [S, H], F32, kind="ExternalInput")
    # weights pre-transposed on host for contiguous DMA:
    # wq/wk: [kc, p, hc, j]; wv/wo: [p, hc|kvc, j] (all fp8, x64 scaled)
    wq_d = nc.dram_tensor("wq", [NCHK, 128, NCHK, 128], F8, kind="ExternalInput")
    wk_d = nc.dram_tensor("wk", [NCHK, 128, NCHK, 128], F8, kind="ExternalInput")
    wv_d = nc.dram_tensor("wv", [128, NCHK, KEY], F8, kind="ExternalInput")
    wo_d = nc.dram_tensor("wo", [128, NCHK, H], F8, kind="ExternalInput")
    gamma_d = nc.dram_tensor("gamma", [128, NCHK], F32, kind="ExternalInput")
    beta_d = nc.dram_tensor("beta", [128, NCHK], F32, kind="ExternalInput")
    mask_d = nc.dram_tensor("mask", [4, 4, 128, 256], F8, kind="ExternalInput")
    y_d = nc.dram_tensor("y", [1024, H], F32, kind="ExternalOutput")
    DBG = bool(os.environ.get("K_DEBUG"))
    if DBG:
        dbg_s = nc.dram_tensor("dbg_s", [4, 256], F32, kind="ExternalOutput")
        dbg_q = nc.dram_tensor("dbg_q", [NCHK, 128, 1024], F8, kind="ExternalOutput")
        dbg_k = nc.dram_tensor("dbg_k", [2, 2, NCHK, 128, 512], F8,
                               kind="ExternalOutput")
        dbg_v = nc.dram_tensor("dbg_v", [2, 8, 128, KEY], F8, kind="ExternalOutput")
        dbg_a = nc.dram_tensor("dbg_a", [ATOT, 128, 256], F8, kind="ExternalOutput")
        dbg_o = nc.dram_tensor("dbg_o", [NCHK, 128, 1024], F8, kind="ExternalOutput")
        dbg_xn = nc.dram_tensor("dbg_xn", [NCHK, 128, S], F8, kind="ExternalOutput")
    ssp_d = nc.dram_tensor("ssp", [4, 256], F32, kind="Internal")
    # combined spill buffers per half: rows 0-3 = v chunks, rows 4-7 = k^T
    # tiles packed as [4, 128, 2048] (ks[kc] at row 4+kc//4, cols (kc%4)*512)
    vka_d = nc.dram_tensor("vka", [8, 128, KEY], F8, kind="Internal")
    vkb_d = nc.dram_tensor("vkb", [8, 128, KEY], F8, kind="Internal")
    vkap_d = nc.dram_tensor("vkap", [2, 8, 128, KEY], F8, kind="Internal")
    vkbp_d = nc.dram_tensor("vkbp", [2, 8, 128, KEY], F8, kind="Internal")
    GROUPS = [[2 * p, 2 * p + 1] for p in range(4)]

    ident = nc.inline_tensor(np.eye(128).astype(ml_dtypes.bfloat16), name="ident")

    with tile.TileContext(nc) as tc:
        with (
            tc.tile_pool(name="small", bufs=1) as small,
            tc.tile_pool(name="p_main", bufs=1) as p_main,
        ):
            recip = small.tile([128, 8], F32)         # 1/(4*sums) per q-chunk
            gcol = small.tile([128, NCHK], F32)       # gamma, [p, hc]
            bcol = small.tile([128, NCHK], F32)       # beta
            ones = small.tile([128, 1], F8)           # 4.0 (denominator scale)
            id16_sb = small.tile([128, 128], BF16)

            xnT = p_main.tile([128, NCHK, S], F8)  # x_norm^T [hid_p, hc, local tok]
            qT = p_main.tile([128, NCHK, 1024], F8)  # q^T [key_p, kc, class-packed q]

            nc.vector.memset(ones[:], 4.0)

            # ---------- A0: LN + transpose + v + k (interleaved) ----------
            with (
                nc.named_scope("ln_transpose"),
                tc.tile_pool(name="a0x", bufs=2) as a0x,
                tc.tile_pool(name="a0xp", bufs=8) as a0xp,
                tc.tile_pool(name="a0s", bufs=4) as a0s,
                tc.tile_pool(name="wv", bufs=1) as wvp,
                tc.tile_pool(name="vst", bufs=2) as vst,
                tc.tile_pool(name="kst", bufs=2) as kst,
                tc.tile_pool(name="pp_tr", bufs=2, space=bass.MemorySpace.PSUM) as pp_tr,
                tc.tile_pool(name="pp_v", bufs=2, space=bass.MemorySpace.PSUM) as pp_v,
                tc.tile_pool(name="pp_k", bufs=2, space=bass.MemorySpace.PSUM) as pp_k,
                tc.tile_pool(name="pp_q", bufs=2, space=bass.MemorySpace.PSUM) as pp_q,
            ):
                # resident weights, one big DMA each (scalar queue, so the
                # sync queue keeps feeding x chunks without head-of-line wait)
                wv_sb = wvp.tile([128, NCHK, KEY], F8)
                nc.scalar.dma_start(wv_sb[:], wv_d[:])
                wk_sb = wvp.tile([128, NCHK, NCHK, 128], F8)   # [p, kc, hc, j]
                nc.scalar.dma_start(wk_sb[:],
                                    wk_d.rearrange("kc p hc j -> p kc hc j"))
                wq_sb = wvp.tile([128, NCHK, NCHK, 128], F8)
                nc.scalar.dma_start(wq_sb[:],
                                    wq_d.rearrange("kc p hc j -> p kc hc j"))

                def v_pass(lc):
                    """v for own local chunk lc -> combined spill row lc%4."""
                    vs = vst.tile([128, KEY], F8, tag="vs")
                    for kvt in range(4):
                        ps = pp_v.tile([128, 512], F32, tag="v")
                        for hp in range(NCHK // 2):
                            nc.tensor.matmul(
                                ps[:],
                                xnT[:, 2 * hp:2 * hp + 2, lc * 128:(lc + 1) * 128],
                                wv_sb[:, 2 * hp:2 * hp + 2,
                                      kvt * 512:(kvt + 1) * 512],
                                start=(hp == 0), stop=(hp == NCHK // 2 - 1),
                                perf_mode=DR)
                        nc.scalar.activation(vs[:, kvt * 512:(kvt + 1) * 512],
                                             ps[:], AF.Identity,
                                             scale=1.0 / WSCALE)
                    vk = [vka_d, vkb_d][lc // 4]
                    nc.scalar.dma_start(vk[lc % 4][:], vs[:])

                def k_pass(half):
                    """k^T for local tok chunks 4*half..4*half+3 (xnT cols
                    half*512..half*512+512); spills into the combined buffer
                    then gathers v+k for this half in ONE AllGather."""
                    vk = [vka_d, vkb_d][half]
                    gout = [vkap_d, vkbp_d][half]
                    for kc in range(NCHK):
                        psk = pp_k.tile([128, 512], F32, tag="k")
                        for hp in range(NCHK // 2):
                            nc.tensor.matmul(
                                psk[:], wk_sb[:, kc, 2 * hp:2 * hp + 2, :],
                                xnT[:, 2 * hp:2 * hp + 2,
                                    half * 512:(half + 1) * 512],
                                start=(hp == 0), stop=(hp == NCHK // 2 - 1),
                                perf_mode=DR)
                        ks = kst.tile([128, 512], F8, tag="ks")
                        nc.scalar.activation(ks[:], psk[:], AF.Identity,
                                             scale=1.0 / WSCALE)
                        nc.scalar.dma_start(
                            vk[4 + kc // 4, :, (kc % 4) * 512:(kc % 4 + 1) * 512],
                            ks[:])
                    nc.gpsimd.collective_compute(
                        "AllGather", OP.bypass, replica_groups=GROUPS,
                        ins=[vk.ap().opt()], outs=[gout.ap().opt()])

                def q_pass(g):
                    """q^T for xnT cols g*512..(g+1)*512 (classes 2g, 2g+1)."""
                    for kc in range(NCHK):
                        psq = pp_q.tile([128, 512], F32, tag="q")
                        for hp in range(NCHK // 2):
                            nc.tensor.matmul(
                                psq[:], wq_sb[:, kc, 2 * hp:2 * hp + 2, :],
                                xnT[:, 2 * hp:2 * hp + 2, g * 512:(g + 1) * 512],
                                start=(hp == 0), stop=(hp == NCHK // 2 - 1),
                                perf_mode=DR)
                        nc.scalar.activation(qT[:, kc, g * 512:(g + 1) * 512],
                                             psq[:], AF.Identity,
                                             scale=1.0 / WSCALE)

                for tg in range(4):
                    xps = []
                    for i in range(4):
                        tci = tg * 4 + i
                        x_t = a0x.tile([128, H], F32, tag="x")
                        nc.sync.dma_start(x_t[:], x_d[tci * 128:(tci + 1) * 128, :])
                        st = a0s.tile([128, 4, 6], F32, tag="st")
                        for j in range(4):
                            nc.vector.bn_stats(st[:, j, :], x_t[:, j * 512:(j + 1) * 512])
                        ag = a0s.tile([128, 2], F32, tag="ag")
                        nc.vector.bn_aggr(ag[:], st[:])
                        veps = a0s.tile([128, 1], F32, tag="veps")
                        nc.vector.tensor_scalar_add(veps[:], ag[:, 1:2], EPS)
                        sq = a0s.tile([128, 1], F32, tag="sq")
                        nc.scalar.sqrt(sq[:], veps[:])
                        rstd = a0s.tile([128, 1], F32, tag="rstd")
                        nc.vector.reciprocal(rstd[:], sq[:])
                        nmr = a0s.tile([128, 1], F32, tag="nmr")
                        nc.vector.tensor_scalar(nmr[:], ag[:, 0:1], rstd[:], -1.0,
                                                OP.mult, OP.mult)
                        xp = a0xp.tile([128, H], BF16, tag="xp")
                        nc.vector.tensor_scalar(xp[:], x_t[:], rstd[:], nmr[:],
                                                OP.mult, OP.add)
                        xps.append(xp)
                    if tg == 0:
                        # small loads after tg0's x DMAs: x keeps HBM priority
                        nc.sync.dma_start(id16_sb[:], ident[:])
                        nc.sync.dma_start(gcol[:], gamma_d[:])
                        nc.sync.dma_start(bcol[:], beta_d[:])
                    # position chunks tg*4+i: i=0,1 own -> local cols
                    # (2*tg+i)*128; i=2,3 peer -> local cols 1024+(2*tg+i-2)*128
                    for hc in range(NCHK):
                        ps = pp_tr.tile([128, 512], BF16, tag="tr")
                        for i in range(4):
                            nc.tensor.transpose(ps[:, i * 128:(i + 1) * 128],
                                                xps[i][:, hc * 128:(hc + 1) * 128],
                                                id16_sb[:])
                        # one strided eviction: ps cols 0:256 -> own local
                        # cols, 256:512 -> peer cols (base 1024)
                        xr = xnT[:, hc, :].rearrange("p (g t) -> p g t", g=2)
                        nc.scalar.activation(
                            xr[:, :, (2 * tg) * 128:(2 * tg + 2) * 128],
                            ps[:], AF.Identity,
                            bias=bcol[:, hc:hc + 1], scale=gcol[:, hc:hc + 1])
                    # v redistributed: tg0 none (x DMAs get HBM priority while
                    # weights stream); each half's v spills complete before
                    # its k_pass emits the combined v+k AllGather
                    for lc in {0: [], 1: [0, 1, 2, 3], 2: [4, 5], 3: [6, 7]}[tg]:
                        v_pass(lc)
                    if tg == 1 or tg == 3:
                        k_pass(tg // 2)
                        q_pass(tg // 2)

            # ---------- pools for q^T, attn^T, out^T, v-resident ----------
            p_bc0 = tc.tile_pool(name="p_bc", bufs=1)
            p_bc = p_bc0.__enter__()
            aT = p_bc.tile([128, ATOT, 256], F8)      # attn^T tiles
            oT = p_bc.tile([128, NCHK, 1024], F8)     # out^T [kv_p, kvc, q]
            vts = p_bc.tile([128, 16, KEY], F8)       # v^T-source [tok_p, slot, kv]
            wo_sb = p_bc.tile([128, NCHK, H], F8)     # o_proj weights [kv_p, kvc, j]

            # ---------- B+C interleaved: scores -> attn^T; out^T per class ----
            with (
                nc.named_scope("scores"),
                tc.tile_pool(name="bk", bufs=3) as bk,
                tc.tile_pool(name="bm", bufs=4) as bm,
                tc.tile_pool(name="bs", bufs=4) as bs,
                tc.tile_pool(name="pp_s", bufs=2, space=bass.MemorySpace.PSUM) as pp_s,
                tc.tile_pool(name="pp_o", bufs=2, space=bass.MemorySpace.PSUM) as pp_o,
                tc.tile_pool(name="pp_sum", bufs=4,
                             space=bass.MemorySpace.PSUM) as pp_sum,
            ):
                # v resident: [tok_p, slot(r*8+lc), kv]; first half available
                # as soon as the half-0 combined AllGather lands
                for r in range(2):
                    nc.gpsimd.dma_start(
                        vts[:, r * 8:r * 8 + 4, :],
                        vkap_d[r, 0:4].rearrange("lc p j -> p lc j"))
                # o_proj weights prefetch (used in D)
                nc.scalar.dma_start(wo_sb[:], wo_d[:])

                ps_sums = [pp_sum.tile([1, 256], F32, tag="sum", name=f"psum{e}")
                           for e in range(4)]
                pending = []

                def emit_c(e):
                    """out^T class e: needs aT slots lc<=2e+1 (both ranks)."""
                    for kvc in range(NCHK):
                        ps_o = pp_o.tile([128, 256], F32, tag="o")
                        steps = [(r, j) for r in range(2) for j in range(e + 1)]
                        for si, (r, j) in enumerate(steps):
                            nc.tensor.matmul(
                                ps_o[:],
                                vts[:, r * 8 + 2 * j:r * 8 + 2 * j + 2,
                                    kvc * 128:(kvc + 1) * 128],
                                aT[:, ABASE[e] + r * 2 * (e + 1) + 2 * j:
                                   ABASE[e] + r * 2 * (e + 1) + 2 * j + 2, :],
                                start=(si == 0), stop=(si == len(steps) - 1),
                                perf_mode=DR)
                        nc.scalar.activation(oT[:, kvc, e * 256:(e + 1) * 256],
                                             ps_o[:], AF.Identity, scale=1.0 / 16)

                # slot (r, lc): rank r's local tok chunk lc; tile-grouped by
                # (half, r) with 4 lc each. Class e consumes lc < 2*(e+1).
                for half in range(2):
                    src = [vkap_d, vkbp_d][half]
                    if half == 1:
                        for r in range(2):
                            nc.gpsimd.dma_start(
                                vts[:, r * 8 + 4:r * 8 + 8, :],
                                vkbp_d[r, 0:4].rearrange("lc p j -> p lc j"))
                    for r in range(2):
                        # k^T tiles: rows 4-7 of the combined gather, packed
                        # [q][p][(k j)] with kc = q*4 + k
                        ktq4 = bk.tile([128, 4, 4, 512], F8, tag="kt")
                        nc.gpsimd.dma_start(
                            ktq4[:], src[r, 4:8].rearrange(
                                "q p (k j) -> p q k j", j=512))
                        ktq = ktq4[:].rearrange("p q k j -> p (q k) j")
                        for lcc in range(4):
                            lc = half * 4 + lcc
                            this_round = []
                            for e in range(lc // 2, 4):
                                ps_s = pp_s.tile([128, 256], F32, tag="s")
                                for kp in range(NCHK // 2):
                                    nc.tensor.matmul(
                                        ps_s[:],
                                        ktq[:, 2 * kp:2 * kp + 2,
                                            lcc * 128:(lcc + 1) * 128],
                                        qT[:, 2 * kp:2 * kp + 2,
                                           e * 256:(e + 1) * 256],
                                        start=(kp == 0), stop=(kp == NCHK // 2 - 1),
                                        perf_mode=DR)
                                dst = aT[:, ABASE[e] + r * 2 * (e + 1) + lc, :]
                                if lc // 2 == e:
                                    tmp = bs.tile([128, 256], F8, tag="exps")
                                    nc.scalar.activation(tmp[:], ps_s[:], AF.Exp,
                                                         scale=SCALE)
                                    mt = bm.tile([128, 256], F8, tag="mask")
                                    nc.sync.dma_start(mt[:], mask_d[e, r * 2 + lc % 2])
                                    nc.vector.tensor_mul(dst, tmp[:], mt[:])
                                else:
                                    nc.scalar.activation(dst, ps_s[:], AF.Exp,
                                                         scale=SCALE)
                                this_round.append((e, (r, lc), dst))
                            for e, pos, src2 in pending:
                                nc.tensor.matmul(ps_sums[e][:], ones[:, 0:1], src2,
                                                 start=(pos == (0, 0)),
                                                 stop=(pos == (1, 2 * e + 1)))
                            pending = this_round
                            if r == 1 and lc % 2 == 1:
                                ecl = lc // 2    # class ecl complete
                                for e, pos, src2 in pending:
                                    nc.tensor.matmul(
                                        ps_sums[e][:], ones[:, 0:1], src2,
                                        start=(pos == (0, 0)),
                                        stop=(pos == (1, 2 * e + 1)))
                                pending = []
                                emit_c(ecl)
                for e in range(4):
                    srow = bs.tile([1, 256], F32, tag="srow", name=f"srow{e}")
                    nc.scalar.copy(srow[:], ps_sums[e][:])
                    nc.scalar.dma_start(ssp_d[e], srow[:])
                for e in range(4):
                    scol = bs.tile([128, 2], F32, tag="scol", name=f"scol{e}")
                    nc.sync.dma_start(scol[:],
                                      ssp_d[e].rearrange("(j p) -> p j", p=128))
                    nc.vector.reciprocal(recip[:, 2 * e:2 * e + 2], scol[:])

            if DBG:
                nc.sync.dma_start(dbg_s[:], ssp_d[:])
                for r in range(2):
                    for q in range(4):
                        nc.sync.dma_start(
                            dbg_k[0, r, q * 4:(q + 1) * 4],
                            vkap_d[r, 4 + q].rearrange("p (k j) -> k p j", j=512))
                        nc.sync.dma_start(
                            dbg_k[1, r, q * 4:(q + 1) * 4],
                            vkbp_d[r, 4 + q].rearrange("p (k j) -> k p j", j=512))
                    nc.sync.dma_start(dbg_v[r, 0:4], vkap_d[r, 0:4])
                    nc.sync.dma_start(dbg_v[r, 4:8], vkbp_d[r, 0:4])
                for kc in range(NCHK):
                    nc.sync.dma_start(dbg_q[kc], qT[:, kc, :])
                    nc.sync.dma_start(dbg_o[kc], oT[:, kc, :])
                    nc.sync.dma_start(dbg_xn[kc], xnT[:, kc, :])
                for t in range(ATOT):
                    nc.sync.dma_start(dbg_a[t], aT[:, t, :])

            # ---------- D: y = diag(recip) (oT^T @ Wo) + x ----------
            with (
                nc.named_scope("o_proj"),
                tc.tile_pool(name="dx", bufs=6) as dx,
                tc.tile_pool(name="dy", bufs=4) as dy,
                tc.tile_pool(name="pp_y", bufs=6, space=bass.MemorySpace.PSUM) as pp_y,
            ):
                for ht in range(4):
                    for qg in range(2):
                        psy = [pp_y.tile([128, 512], F32, tag="y", name=f"psy{i}")
                               for i in range(4)]
                        for kp in range(NCHK // 2):
                            for i in range(4):
                                qc = qg * 4 + i
                                nc.tensor.matmul(
                                    psy[i][:],
                                    oT[:, 2 * kp:2 * kp + 2, qc * 128:(qc + 1) * 128],
                                    wo_sb[:, 2 * kp:2 * kp + 2,
                                          ht * 512:(ht + 1) * 512],
                                    start=(kp == 0), stop=(kp == NCHK // 2 - 1),
                                    perf_mode=DR)
                        for i in range(4):
                            qc = qg * 4 + i
                            xres = dx.tile([128, 512], F32, tag="xr")
                            nc.sync.dma_start(xres[:],
                                              x_d[PC[qc] * 128:(PC[qc] + 1) * 128,
                                                  ht * 512:(ht + 1) * 512])
                            ysb = dy.tile([128, 512], F32, tag="y")
                            nc.vector.scalar_tensor_tensor(
                                ysb[:], psy[i][:], recip[:, qc:qc + 1], xres[:],
                                OP.mult, OP.add)
                            eng = nc.scalar if i % 2 else nc.gpsimd
                            eng.dma_start(y_d[qc * 128:(qc + 1) * 128,
                                              ht * 512:(ht + 1) * 512],
                                          ysb[:])
            p_bc0.__exit__(None, None, None)
    nc.compile()
    return nc


_NC_CACHE = None


def _get_nc():
    global _NC_CACHE
    if _NC_CACHE is None:
        _NC_CACHE = build()
    return _NC_CACHE


def make_in_maps(x, qkv, o_proj, gamma, beta):
    qkv = np.asarray(qkv, dtype=np.float32)
    o_proj = np.asarray(o_proj, dtype=np.float32)
    F8NP = ml_dtypes.float8_e4m3

    def prep_qk(w):  # [H, KEY] -> [kc, p, hc, j] fp8 (x64)
        t = (WSCALE * w).reshape(NCHK, 128, NCHK, 128)  # [hc, p, kc, j]
        return np.ascontiguousarray(t.transpose(2, 1, 0, 3)).astype(F8NP)

    def prep_vo(w):  # [H, KEY] -> [p, hc, j] fp8 (x64)
        t = (WSCALE * w).reshape(NCHK, 128, KEY)        # [hc, p, j]
        return np.ascontiguousarray(t.transpose(1, 0, 2)).astype(F8NP)

    wq8 = prep_qk(qkv[:, :KEY])
    wk8 = prep_qk(qkv[:, KEY:2 * KEY])
    wv8 = prep_vo(qkv[:, 2 * KEY:])
    wo8 = prep_vo(o_proj)
    gamma = np.ascontiguousarray(
        np.asarray(gamma, dtype=np.float32).reshape(NCHK, 128).T)
    beta = np.ascontiguousarray(
        np.asarray(beta, dtype=np.float32).reshape(NCHK, 128).T)
    in_maps, metas = [], []
    for c in range(8):
        b, h = c // 2, c % 2
        P = perm_chunks(h)
        ti = np.concatenate([np.arange(pc * 128, pc * 128 + 128) for pc in P])
        x_perm = np.ascontiguousarray(x[b][ti], dtype=np.float32)
        # mask[e][2r+j]: k tok-slot (rank r, quad e, j) holds true chunk
        # 4e+2r+j; q col c of class e is true row ti[512e+c].
        mask = np.zeros((4, 4, 128, 256), dtype=F8NP)
        for e in range(4):
            qp = ti[512 * e:512 * e + 256]
            for r in range(2):
                for j in range(2):
                    kp = (4 * e + 2 * r + j) * 128 + np.arange(128)
                    mask[e, 2 * r + j] = (kp[:, None] <= qp[None, :]).astype(F8NP)
        in_maps.append({"x": x_perm, "wq": wq8, "wk": wk8, "wv": wv8,
                        "wo": wo8, "gamma": gamma, "beta": beta, "mask": mask})
        metas.append((b, ti))
    return in_maps, metas


def gather(results, metas, dtype):
    out = np.empty((B, S, H), dtype=dtype)
    qpos = np.concatenate([np.arange(512 * e, 512 * e + 256) for e in range(4)])
    for c, (b, ti) in enumerate(metas):
        out[b][ti[qpos]] = results[c]["y"]
    return out


def kernel(x, qkv, o_proj, gamma, beta, _trace=False):
    x = np.asarray(x, dtype=np.float32)
    nc = _get_nc()
    in_maps, metas = make_in_maps(x, qkv, o_proj, gamma, beta)
    res = run_bass_kernel_spmd(nc, in_maps, core_ids=list(range(8)), trace=_trace)
    out = gather(res.results, metas, np.float32)
    if _trace:
        kernel.last_result = res
    return out
